# revision 38
# baseline (speedup 1.0000x reference)
"""Expert-parallel MoE FFN (SwiGLU) for 8 TRN2 NeuronCores.

Strategy: expert parallelism. Host sorts tokens by expert_id, pads each
expert's token group to a common capacity C, and ships core e: its
expert's weights (bf16) + its tokens transposed (bf16). Each core runs
a dense SwiGLU FFN for its expert in transposed layout (features on
partitions, tokens on the moving axis), so both weight matrices are
used directly as the stationary matmul operand with no on-device
transposes. Host unpermutes the per-core outputs.

The schedule targets the bf16 TensorE roofline (384*C cycles/core):
- Host packs every input tensor in the exact SBUF tile layout
  ([128 partitions, k-major columns], chunk-major for x), so each load
  is ONE 2D-contiguous DMA with 8KB runs: ~10x fewer descriptors than
  a strided 3D access pattern, ~1 us descriptor generation, full-rate
  transfer.
- All weight DMAs stream on the single gpsimd ring in need-order; x
  rides the sync HWDGE ring. The DMA hardware fair-shares bandwidth
  between rings at packet granularity, so need-order must be enforced
  within one ring per traffic class (measured: spreading weights over
  two rings let later-needed transfers starve earlier-needed ones for
  a 21 us PE stall).
- Warm-up matmuls on a zeroed tile bridge the head DMA fill so the PE
  HAM clock gate is already at full rate when real matmuls start.
"""

import numpy as np
import ml_dtypes

import concourse.bass as bass  # noqa: F401
import concourse.tile as tile
from concourse import bacc, mybir
from concourse import bass_utils

H = 1024
F = 2048
F2 = 2 * F
E = 8
N_CORES = 8
P = 128
KH = H // P      # 8  k-chunks for matmul 1
KF = F // P      # 16 k-chunks for matmul 2
NJ = F // P      # 16 gate/up feature-chunk pairs
NHOUT = H // P   # 8  output row chunks
# wgu column segments (one DMA per segment, need-order); the whole gate
# half is narrow (256-col, 0.5 MB links) so every chained link lands
# ahead of the gate sweep's demand even when x0's completion is late;
# the up half has ~15+ us of slack and ships in 512-col links.
SEG_WIDTHS = [256] * 8 + [512] * 4
TOK_CHUNK = 512
# HAM warm-up matmuls bridging the head to x0's completion semaphore
# (measured 15.5-17.3 us, receipt-bound). Two phases self-normalize the
# bridge length: the N=128 phase covers the ~3.4 us HAM warm window
# (whatever the starting clock), the N=256 phase then runs at the warm
# clock, ending ~15.1 us regardless of the initial HAM state.
NDUMMY1 = 40     # N=128
NDUMMY2 = 20     # N=256 — cold-start bridge ends ~13.3 us, just under the
                 # last x0 sub-DMA semaphore (~13.6, issue-rate bound)

BF16 = mybir.dt.bfloat16
F32 = mybir.dt.float32

_KERNEL_CACHE = {}


def _chunk_sizes(capacity: int):
    # Token chunk sizes (the token axis is the matmul moving/free dim, so
    # no 128 alignment needed). A full 512 first chunk keeps the weight
    # chain's demand spacing wide (x0's completion is receipt-latency
    # bound, so a smaller first chunk doesn't shorten the head); keep
    # every chunk >= 256: short matmuls can't hide per-accumulation-group
    # boundary stalls.
    if capacity <= TOK_CHUNK:
        return [capacity]
    sizes = []
    rem = capacity
    while rem > 2 * TOK_CHUNK:
        sizes.append(TOK_CHUNK)
        rem -= TOK_CHUNK
    if rem > 768:            # (768, 1024]
        sizes += [TOK_CHUNK, rem - TOK_CHUNK]
    elif rem > 512:          # (512, 768]
        sizes += [rem - 256, 256]
    else:
        sizes += [rem]
    return sizes


def _segs():
    segs = []
    off = 0
    for w in SEG_WIDTHS:
        segs.append((off, w))
        off += w
    assert off == F2
    return segs


def _build(capacity: int):
    """Build + compile the per-core SPMD kernel for token capacity C."""
    nc = bacc.Bacc(
        "TRN2",
        target_bir_lowering=False,
        debug=False,
        num_devices=N_CORES,
    )
    # Inputs are host-packed to the SBUF layout:
    #   xt:  [128, KH*C], chunk-major: chunk ci at cols KH*t0ci + k*ntci + t
    #   wgu: [128, sum(KH*w)] seg-major, k-major inside a segment
    #   wd:  [128, 2*8*H] two f-halves, f-major inside a half
    xt_d = nc.dram_tensor("xt", [P, KH * capacity], BF16, kind="ExternalInput").ap()
    wgu_d = nc.dram_tensor("wgu", [P, KH * F2], BF16, kind="ExternalInput").ap()
    wd_d = nc.dram_tensor("wd", [P, KF * H], BF16, kind="ExternalInput").ap()
    # output, chunk-major packed like x: chunk ci occupies cols
    # [NHOUT*t0, NHOUT*(t0+nt)), hh-major inside (host unpacks)
    yt_d = nc.dram_tensor("yt", [P, NHOUT * capacity], BF16, kind="ExternalOutput").ap()

    sizes = _chunk_sizes(capacity)
    chunks = []
    t0 = 0
    for nt in sizes:
        chunks.append((t0, nt))
        t0 += nt
    segs = _segs()

    def seg_of(col):
        for si, (s0, w) in enumerate(segs):
            if s0 <= col < s0 + w:
                return si, col - s0
        raise AssertionError(col)

    with tile.TileContext(nc) as tc:
        with (
            tc.tile_pool(name="weights", bufs=1) as wpool,
            tc.tile_pool(name="xin", bufs=2) as xpool,
            tc.tile_pool(name="hid", bufs=2) as hpool,
            tc.tile_pool(name="gat", bufs=1) as gpool,
            tc.tile_pool(name="yout", bufs=2) as ypool,
            tc.tile_pool(name="dum", bufs=1) as dpool,
            tc.tile_pool(name="ps1", bufs=2, space="PSUM") as ps1,
            tc.tile_pool(name="ps2", bufs=3, space="PSUM") as ps2,
            tc.tile_pool(name="psd", bufs=1, space="PSUM") as psd,
        ):
            # --- HAM warm-up: dummy matmuls on zeroed tiles keep the PE
            # busy (and its clock gate open) while the head DMAs land.
            wz = dpool.tile([P, P], BF16, name="wz", tag="wz")
            wz2 = dpool.tile([P, 2 * P], BF16, name="wz2", tag="wz2")
            nc.vector.memset(wz[:], 0.0)
            nc.vector.memset(wz2[:], 0.0)
            dps = psd.tile([P, 2 * P], F32, name="dps", tag="dps")
            for _ in range(NDUMMY1):
                nc.tensor.matmul(dps[:, 0:P], wz[:], wz[:], start=True, stop=True)
            for _ in range(NDUMMY2):
                nc.tensor.matmul(dps[:], wz[:], wz2[:], start=True, stop=True)

            # --- head DMAs. Gate-group-0's complete dependency set (wgu
            # segments 0a/0b, then x chunk 0) rides the sync HWDGE ring:
            # ~1.4 us to first byte vs ~5 us pre-transfer + ~2 us
            # completion latency on the gpsimd SWDGE path (measured).
            wgu_sb = []
            for si, (s0, w) in enumerate(segs):
                wt = wpool.tile([P, KH * w], BF16, name=f"wgu{si}", tag=f"wgu{si}")
                wgu_sb.append(wt)
                if si < 2:
                    nc.sync.dma_start(wt[:], wgu_d[:, KH * s0:KH * (s0 + w)])

            # x chunk 0 ships as KH per-k sub-DMAs: completion semaphores
            # fire progressively (~0.35 us apart) and the small receipts
            # pipeline behind later transfers, so the first gate group
            # completes ~4 us earlier than with one 1 MB DMA whose single
            # semaphore waits for the whole window to drain (measured).
            # Subtile deps let each k-matmul wait only on its own slice.
            t0, nt = chunks[0]
            x_sb = xpool.tile([P, KH * nt], BF16, name="x", tag="x")
            for k in range(KH):
                nc.sync.dma_start(
                    x_sb[:, k * nt:(k + 1) * nt],
                    xt_d[:, KH * t0 + k * nt:KH * t0 + (k + 1) * nt],
                )

            # Chain every later DMA behind x0, two transfers in flight
            # (depth-2): full aggregate bandwidth, but each transfer's
            # completion semaphore fires as soon as its own bytes land.
            # Measured mechanisms that force this: (a) concurrent queues
            # fair-share DMA bandwidth, so an open flood halves the
            # head-critical rate; (b) descriptors of many DMAs in flight
            # interleave across the 16 SDMA engines, so completion
            # semaphores only fire when the whole window drains; (c) a
            # depth-1 chain pays ~2-3 us receipt latency per link and
            # starves the tail of the stream. A priority hint is not
            # enough (the scheduler hoists ready DMAs), so each link is
            # a real dependency: a 4-element copy from the
            # two-transfers-back destination into the next before its
            # dma_start.
            wd_sb = []
            for w2 in range(2):
                wd_sb.append(
                    wpool.tile([P, 8 * H], BF16, name=f"wd{w2}", tag=f"wd{w2}")
                )
            x1_tile = None
            if len(chunks) > 1:
                t01, nt1 = chunks[1]
                x1_tile = xpool.tile([P, KH * nt1], BF16, name="x", tag="x")

            # last two link destinations; the x0 anchor is its LAST per-k
            # slice so the chain waits for the final x0 sub-DMA
            x_last = x_sb[:, (KH - 1) * nt:KH * nt]
            chain_tail = [x_last, x_last]

            def chained_dma(eng, dst_tile, src_ap):
                nc.gpsimd.tensor_copy(dst_tile[:, 0:4], chain_tail[0][:, 0:4])
                eng.dma_start(dst_tile[:], src_ap)
                chain_tail[0] = chain_tail[1]
                chain_tail[1] = dst_tile

            # need-order: gate segs, x1, up segs, then wd halves.
            for si in range(2, 8):
                s0, w = segs[si]
                chained_dma(nc.gpsimd, wgu_sb[si], wgu_d[:, KH * s0:KH * (s0 + w)])
            if x1_tile is not None:
                t01, nt1 = chunks[1]
                chained_dma(nc.sync, x1_tile, xt_d[:, KH * t01:KH * (t01 + nt1)])
            for si in range(8, len(segs)):
                s0, w = segs[si]
                chained_dma(nc.gpsimd, wgu_sb[si], wgu_d[:, KH * s0:KH * (s0 + w)])
            for w2 in range(2):
                chained_dma(
                    nc.gpsimd, wd_sb[w2], wd_d[:, w2 * 8 * H:(w2 + 1) * 8 * H]
                )

            for ci, (t0, nt) in enumerate(chunks):
                # gate pass: G_j = silu(sum_k wgu[k, j].T @ x[k]) into SBUF f32
                g_sb = []
                for j in range(NJ):
                    si, co = seg_of(j * P)
                    w = segs[si][1]
                    g_ps = ps1.tile([P, nt], F32, name=f"g{j}", tag="g")
                    for k in range(KH):
                        nc.tensor.matmul(
                            g_ps[:],
                            wgu_sb[si][:, k * w + co:k * w + co + P],
                            x_sb[:, k * nt:(k + 1) * nt],
                            start=(k == 0),
                            stop=(k == KH - 1),
                        )
                    gt = gpool.tile([P, nt], F32, name=f"gt{j}", tag=f"gt{j}")
                    nc.scalar.activation(
                        gt[:], g_ps[:], mybir.ActivationFunctionType.Silu
                    )
                    g_sb.append(gt)

                # Prefetch the next chunk's x (single contiguous descriptor).
                # x1 was issued inside the head chain; later prefetches are
                # naturally held back by the x pool ring reuse.
                if ci + 1 < len(chunks):
                    if ci == 0:
                        x_next = x1_tile
                    else:
                        t0n, ntn = chunks[ci + 1]
                        x_next = xpool.tile([P, KH * ntn], BF16, name="x", tag="x")
                        nc.sync.dma_start(
                            x_next[:], xt_d[:, KH * t0n:KH * (t0n + ntn)]
                        )

                # up pass: hidden_j = G_j * (sum_k wgu[k, 16+j].T @ x[k]) bf16
                h_sb = []
                for j in range(NJ):
                    si, co = seg_of((NJ + j) * P)
                    w = segs[si][1]
                    u_ps = ps1.tile([P, nt], F32, name=f"u{j}", tag="u")
                    for k in range(KH):
                        nc.tensor.matmul(
                            u_ps[:],
                            wgu_sb[si][:, k * w + co:k * w + co + P],
                            x_sb[:, k * nt:(k + 1) * nt],
                            start=(k == 0),
                            stop=(k == KH - 1),
                        )
                    ht = hpool.tile([P, nt], BF16, name=f"h{j}", tag=f"h{j}")
                    nc.vector.tensor_mul(ht[:], g_sb[j][:], u_ps[:])
                    h_sb.append(ht)

                # down projection: y_hh = sum_f wd[f, hh].T @ h[f]
                last = ci == len(chunks) - 1
                y_sb = ypool.tile([P, NHOUT * nt], BF16, name="y", tag="y")
                for hh in range(NHOUT):
                    y_ps = ps2.tile([P, nt], F32, name=f"y{hh}", tag="yp")
                    for f in range(KF):
                        w2, fi = divmod(f, 8)
                        nc.tensor.matmul(
                            y_ps[:],
                            wd_sb[w2][:, fi * H + hh * P:fi * H + hh * P + P],
                            h_sb[f][:],
                            start=(f == 0),
                            stop=(f == KF - 1),
                        )
                    nc.scalar.copy(y_sb[:, hh * nt:(hh + 1) * nt], y_ps[:])
                    if last and hh == NHOUT - 2:
                        # tail trim: ship the last chunk's first 7 output
                        # rows while hh=7 is still in the pipeline
                        nc.scalar.dma_start(
                            yt_d[:, NHOUT * t0:NHOUT * t0 + (NHOUT - 1) * nt],
                            y_sb[:, :(NHOUT - 1) * nt],
                        )
                # one packed y DMA per chunk on the scalar HWDGE ring —
                # keeps the sync ring free for x and shortens the tail.
                if last:
                    nc.scalar.dma_start(
                        yt_d[:, NHOUT * t0 + (NHOUT - 1) * nt:NHOUT * (t0 + nt)],
                        y_sb[:, (NHOUT - 1) * nt:],
                    )
                else:
                    nc.scalar.dma_start(
                        yt_d[:, NHOUT * t0:NHOUT * (t0 + nt)], y_sb[:]
                    )

                if ci + 1 < len(chunks):
                    x_sb = x_next

    nc.compile()
    return nc


def _get_kernel(capacity: int):
    if capacity not in _KERNEL_CACHE:
        _KERNEL_CACHE[capacity] = _build(capacity)
    return _KERNEL_CACHE[capacity]


def _pack_rows(a, kb):
    """[kb*128, N] f32/bf16 -> [128, kb*N]: row k*128+p, col c -> (p, k*N+c)."""
    kbp, n = a.shape
    assert kbp == kb * P
    return np.ascontiguousarray(
        a.reshape(kb, P, n).transpose(1, 0, 2).reshape(P, kb * n)
    )


def kernel(tokens, w_gate_up, w_down, expert_ids, _run_opts=None):
    tokens = np.asarray(tokens, dtype=np.float32)
    w_gate_up = np.asarray(w_gate_up, dtype=np.float32)
    w_down = np.asarray(w_down, dtype=np.float32)
    eids = np.asarray(expert_ids).astype(np.int64)
    n_tok = tokens.shape[0]

    counts = np.bincount(eids, minlength=E)
    # exact max expert count — the token axis is the matmul moving/free
    # dim everywhere, so capacity needs no 128 alignment
    capacity = int(max(counts.max(), 8))
    sizes = _chunk_sizes(capacity)

    order = np.argsort(eids, kind="stable")
    bf = ml_dtypes.bfloat16

    in_maps = []
    starts = np.zeros(E + 1, dtype=np.int64)
    np.cumsum(counts, out=starts[1:])
    for e in range(E):
        idx = order[starts[e]:starts[e + 1]]
        xe = np.zeros((capacity, H), dtype=np.float32)
        xe[: len(idx)] = tokens[idx]
        xeT = np.ascontiguousarray(xe.T).astype(bf)  # [H, C]
        # chunk-major pack: chunk ci block [128, KH*nt], k-major inside
        xt = np.empty((P, KH * capacity), dtype=bf)
        t0 = 0
        for nt in sizes:
            xt[:, KH * t0:KH * (t0 + nt)] = _pack_rows(xeT[:, t0:t0 + nt], KH)
            t0 += nt
        # wgu: seg-major pack
        wgu_e = w_gate_up[e].astype(bf)  # [H, 2F]
        wgu = np.empty((P, KH * F2), dtype=bf)
        off = 0
        for s0, w in zip(np.cumsum([0] + SEG_WIDTHS[:-1]), SEG_WIDTHS):
            wgu[:, KH * off:KH * (off + w)] = _pack_rows(wgu_e[:, s0:s0 + w], KH)
            off += w
        # wd: two f-halves
        wd_e = w_down[e].astype(bf)  # [F, H]
        wd = np.empty((P, KF * H), dtype=bf)
        wd[:, :8 * H] = _pack_rows(wd_e[:8 * P, :], 8)
        wd[:, 8 * H:] = _pack_rows(wd_e[8 * P:, :], 8)
        in_maps.append({"xt": xt, "wgu": wgu, "wd": wd})

    nc = _get_kernel(capacity)
    run_kwargs = dict(_run_opts or {})
    res = bass_utils.run_bass_kernel_spmd(
        nc, in_maps, core_ids=list(range(N_CORES)), **run_kwargs
    )

    out = np.zeros((n_tok, H), dtype=np.float32)
    for e in range(E):
        idx = order[starts[e]:starts[e + 1]]
        ytp = res.results[e]["yt"]  # [128, NHOUT*capacity] bf16, chunk-major
        # unpack to [H, capacity]
        yt = np.empty((H, capacity), dtype=np.float32)
        t0 = 0
        for nt in sizes:
            blk = ytp[:, NHOUT * t0:NHOUT * (t0 + nt)].astype(np.float32)
            yt[:, t0:t0 + nt] = (
                blk.reshape(P, NHOUT, nt).transpose(1, 0, 2).reshape(H, nt)
            )
            t0 += nt
        out[idx] = yt[:, : len(idx)].T
    if run_kwargs.get("trace"):
        kernel.last_exec_time_ns = res.exec_time_ns
        kernel.last_results = res
    return out
